# revision 35
# baseline (speedup 1.0000x reference)
"""Trainium2 Bass kernel for LocalXLAttention (chunk-summed variant).

Math: the reference einsum sums over the chunk index z, so every query
attends to the same three [w, dh] K/V matrices built from chunk sums:
  K_prev = S_k - k_chunk[C-1], K_cur = S_k, K_next = S_k - k_chunk[0]
(identically for V), where S_k = sum_c k_chunk[c].  Per position l, head h:
  attn[l,h,:]  = qp[l,h,:] @ KbigT          (KbigT: [dh, 3w])
  probs        = softmax(attn, axis=-1)
  ctx[l,h,:]   = probs[l,h,:] @ Vbig        (Vbig:  [3w, dh])
  out          = ctx.reshape(L, dm) @ Wc

Sharding: L=4096 split 512 rows/core across 8 cores (data-parallel over
sequence, no collectives).  All inputs are bf16 (host-cast; halves DMA,
same 1-cycle/row PE rate as fp32r).

Schedule notes (engines execute their queues in order, so emission order
IS the schedule):
 - The Scalar engine's exp stream is the critical resource (~1.2us per
   [128,1024] tile, 96 tiles).  The attention loop is software-pipelined:
   QK of step s+1 is emitted right after exp(s); PV runs TWO steps behind
   exp so ctx-psum slot reuse never blocks the PE queue.
 - Prologue PE work is minimized: the kv chunk-sum runs as an in-place
   bf16 tree on the (otherwise idle) DVE, so the S/c0/c7 projections are
   24 matmuls instead of 144; QP head-blocks 0/1 accumulate in the two
   psmm slots during the kv stream; blocks 2/3 are drip-fed into the
   attention loop's PE slack during pairs 0 and 2.
 - Softmax normalization is deferred (an all-ones Vbig column accumulates
   the denominator); per pair: reciprocal_approx_fast on DVE, a
   DRAM-bounce broadcast DMA, and in-place DVE multiplies -- all staged
   across the next pair's steps, nothing on the PE queue.  The last pair
   normalizes via a tiny fp32 K=1 outer-product matmul instead (PE is
   free then and it avoids the DMA round-trip latency in the tail).
"""

import sys
for _p in ('/opt/pypackages', '/opt/trn_rl_repo'):
    if _p not in sys.path:
        sys.path.insert(0, _p)

import math
import numpy as np
import ml_dtypes

import concourse.bass as bass
import concourse.bacc as bacc
import concourse.tile as tile
from concourse import mybir
from concourse.bass_utils import run_bass_kernel_spmd
from concourse.masks import make_identity

F32 = mybir.dt.float32
BF16 = mybir.dt.bfloat16
I16 = mybir.dt.int16
AF = mybir.ActivationFunctionType
ALU = mybir.AluOpType

# Schraudolph fast-exp: bf16 bits of exp(0.125*x) ~= round(A*x + B).
# One DVE tensor_scalar per tile (vs ~1.25us of ScalarE exp); j-chunks in
# FAST_J use it, spreading softmax work across both engines.  Max rel err
# ~3.3% per element, which washes out to <2e-3 in the output (measured).
FAST_J = (3, 7, 11)
FAST_A = 0.125 * 128 / math.log(2)
FAST_B = 127.0 * 128 - 5.5

N_CORES = 8
L = 4096          # full sequence
LS = L // N_CORES # 512 rows per core
DM = 1024
NH = 16
DH = 64
W = 512           # chunk width
C = L // W        # 8 chunks
J3 = 3 * W        # 1536 softmax width
NJ = J3 // 128    # 12 j-chunks
DMT = DM // 128   # 8 dm-chunks
NPAIR = 8         # head pairs
NSTEP = NPAIR * NJ


def build_nc():
    nc = bacc.Bacc(None, target_bir_lowering=False)

    # all inputs host-packed [1024, N] -> [128, 8, N]: partition p of slice d
    # holds source row 128*d + p, so one big-descriptor DMA loads everything
    qT = nc.dram_tensor("qT", [128, DMT, LS], BF16, kind="ExternalInput")
    kvT = nc.dram_tensor("kvT", [128, DMT, L], BF16, kind="ExternalInput")
    Wq = nc.dram_tensor("Wq", [128, DMT, DM], BF16, kind="ExternalInput")
    Wkv = nc.dram_tensor("Wkv", [128, DMT, 2 * DH], BF16,
                         kind="ExternalInput")
    Wc = nc.dram_tensor("Wc", [128, DMT, DM], BF16, kind="ExternalInput")
    out = nc.dram_tensor("out", [LS, DM], F32, kind="ExternalOutput")

    with tile.TileContext(nc) as tc:
        with tc.tile_pool(name="weights", bufs=8) as wpool, \
             tc.tile_pool(name="small", bufs=1) as spool, \
             tc.tile_pool(name="qp", bufs=8) as qpool, \
             tc.tile_pool(name="qpt", bufs=4) as qptpool, \
             tc.tile_pool(name="stream", bufs=1) as stpool, \
             tc.tile_pool(name="ksum", bufs=2) as kspool, \
             tc.tile_pool(name="probs", bufs=6) as ppool, \
             tc.tile_pool(name="norm", bufs=4) as npool, \
             tc.tile_pool(name="bcast", bufs=2) as bcpool, \
             tc.tile_pool(name="misc", bufs=2) as mpool, \
             tc.tile_pool(name="dram", bufs=2, space="DRAM") as dpool, \
             tc.tile_pool(name="psacc", bufs=4, space="PSUM") as psacc, \
             tc.tile_pool(name="psmm", bufs=2, space="PSUM") as psmm:

            # ---------- warm the exp activation table early ----------------
            dummy = spool.tile([1, 8], F32, tag="dummy")
            nc.vector.memset(dummy, 0.0)
            nc.scalar.activation(dummy, dummy, AF.Exp, scale=0.125)

            # ---------- DMA issues: one big transfer per input -------------
            # (16KB-per-partition descriptors move ~3x the bytes/sec of the
            # per-d 2-4KB ones).  Priority: wkv, wq, qt on gpsimd -- the QP
            # matmuls need them early; kv in 4 transfers on sync+scalar.
            wkv_all = wpool.tile([128, DMT, 2 * DH], BF16, tag="wkv", bufs=1)
            nc.gpsimd.dma_start(out=wkv_all, in_=Wkv[:, :, :])
            wq_all = wpool.tile([128, DMT, DM], BF16, tag="wq", bufs=1)
            nc.gpsimd.dma_start(out=wq_all, in_=Wq[:, :, :])
            qt_all = qpool.tile([128, DMT, LS], BF16, tag="qt", bufs=1)
            nc.gpsimd.dma_start(out=qt_all, in_=qT[:, :, :])
            wkv_sb = [wkv_all[:, d, :] for d in range(DMT)]
            wq_sb = [wq_all[:, d, :] for d in range(DMT)]
            qt_sb = [qt_all[:, d, :] for d in range(DMT)]

            ident = spool.tile([128, 128], F32, tag="ident")
            make_identity(nc, ident)
            ones_sb = spool.tile([1, 128], F32, tag="ones")
            nc.vector.memset(ones_sb, 1.0)

            # Vbig shell + its ones (denominator) column, built while DMAs run
            vbig = spool.tile([128, NJ, 68], BF16, tag="vbig")
            ones_col = spool.tile([128, 1], F32, tag="onescol")
            nc.vector.memset(ones_col, 1.0)
            for j in range(NJ):
                nc.vector.tensor_copy(vbig[:, j, DH:DH + 1], ones_col)

            # ---------- kv stream -> DVE chunk-sum tree -> projections -----
            # ps_S = Wkv.T @ (sum_c kv_chunk_c); ps_0/ps_7 = chunk 0/7 proj.
            # rows 0:64 = K, rows 64:128 = V (full-M packed matmuls).
            # QP head-blocks 0,1 ([128,1024] psum each) ride along per-d in
            # the two psmm slots.
            ps_S = psacc.tile([128, W], F32, tag="acc", name="ps_S")
            ps_0 = psacc.tile([128, W], F32, tag="acc", name="ps_0")
            ps_7 = psacc.tile([128, W], F32, tag="acc", name="ps_7")
            qp01_ps = [psmm.tile([128, 1024], F32, tag="mm", name=f"qp_ps{g}")
                       for g in range(2)]

            qp_half_ps = {}
            qpt_sb = [None] * 4

            def emit_qp23_half_mms(t4, half, ds):
                """attention-injected QP head-block (t4 in {2,3}): one
                [128, LS] psacc half, accumulated over the given d's."""
                key = (t4, half)
                if key not in qp_half_ps:
                    qp_half_ps[key] = psacc.tile(
                        [128, W], F32, tag="acc", name=f"qph{t4}_{half}")
                ps = qp_half_ps[key]
                hd = 2 * t4 + half
                for d in ds:
                    nc.tensor.matmul(ps, wq_sb[d][:, 128 * hd:128 * (hd + 1)],
                                     qt_sb[d], start=(d == 0),
                                     stop=(d == DMT - 1))

            def emit_qp23_half_copy(t4, half):
                if qpt_sb[t4] is None:
                    qpt_sb[t4] = qptpool.tile([128, 1024], BF16, tag="qpt",
                                              name=f"qpt{t4}")
                nc.vector.tensor_copy(
                    qpt_sb[t4][:, 512 * half:512 * (half + 1)],
                    qp_half_ps.pop((t4, half)))

            # kv in 4 transfers of 2 d-slices each (16KB/partition)
            kv_all = stpool.tile([128, DMT, L], BF16, tag="kvstream")
            for g4 in range(4):
                eng = nc.sync if g4 % 2 == 0 else nc.scalar
                eng.dma_start(out=kv_all[:, 2 * g4:2 * g4 + 2, :],
                              in_=kvT[:, 2 * g4:2 * g4 + 2, :])
            st_sb = [kv_all[:, d, :] for d in range(DMT)]

            # per-d: QP blocks 0/1 first (gated only on wq/qt), then the
            # kv-gated c0/c7 projections, DVE tree-sum, and S projection
            for d in range(DMT):
                st = st_sb[d]
                for g in range(2):
                    for half in range(2):
                        hd = 2 * g + half
                        nc.tensor.matmul(
                            qp01_ps[g][:, 512 * half:512 * (half + 1)],
                            wq_sb[d][:, 128 * hd:128 * (hd + 1)],
                            qt_sb[d], start=(d == 0), stop=(d == DMT - 1))
                nc.tensor.matmul(ps_0, wkv_sb[d], st[:, 0:W],
                                 start=(d == 0), stop=(d == DMT - 1))
                nc.tensor.matmul(ps_7, wkv_sb[d], st[:, L - W:L],
                                 start=(d == 0), stop=(d == DMT - 1))
                # in-place bf16 tree: chunk sum (c0 slice is read by the ps_0
                # matmul first; c7 slice is never written)
                nc.vector.tensor_add(st[:, 0:2048], st[:, 0:2048],
                                     st[:, 2048:4096])
                nc.vector.tensor_add(st[:, 0:1024], st[:, 0:1024],
                                     st[:, 1024:2048])
                ks = kspool.tile([128, W], BF16, tag="ks", name=f"ks{d}")
                nc.vector.tensor_add(ks, st[:, 0:512], st[:, 512:1024])
                if d == DMT - 1:
                    # QP blocks 0/1 are complete before the last S matmul;
                    # copy them out now so the first QK isn't queued behind it
                    for g in range(2):
                        qpt_sb[g] = qptpool.tile([128, 1024], BF16, tag="qpt",
                                                 name=f"qpt{g}")
                        nc.vector.tensor_copy(qpt_sb[g], qp01_ps[g])
                nc.tensor.matmul(ps_S, wkv_sb[d], ks,
                                 start=(d == 0), stop=(d == DMT - 1))

            # ---------- Kbig [128, 1536] = [prev | cur | next] (bf16) ------
            s_sb = spool.tile([128, W], F32, tag="ssb")
            nc.vector.tensor_copy(s_sb, ps_S)
            kbig = spool.tile([128, J3], BF16, tag="kbig")
            nc.vector.tensor_sub(kbig[0:DH, 0:W], s_sb[0:DH, :], ps_7[0:DH, :])
            nc.vector.tensor_copy(kbig[0:DH, W:2 * W], s_sb[0:DH, :])
            nc.vector.tensor_sub(kbig[0:DH, 2 * W:3 * W], s_sb[0:DH, :],
                                 ps_0[0:DH, :])
            nc.vector.tensor_copy(kbig[DH:128, :], kbig[0:DH, :])

            # V variants in [dh, l] layout (f32, for PE transpose)
            vprev = spool.tile([DH, W], F32, tag="vprev")
            nc.vector.tensor_sub(vprev, s_sb[DH:128, :], ps_7[DH:128, :])
            vnext = spool.tile([DH, W], F32, tag="vnext")
            nc.vector.tensor_sub(vnext, s_sb[DH:128, :], ps_0[DH:128, :])
            vcur = s_sb[DH:128, :]

            # ---------- Vbig payload: 12 PE transposes -> bf16 copies ------
            # (emitted inside the attention loop's first step so the first
            # QK/exp isn't queued behind them; PV first needs vbig at s=2)
            def emit_vbig_payload():
                for vi, vsrc in enumerate((vprev, vcur, vnext)):
                    # vcur is a slice of s_sb at partition base 64; use the
                    # matching diagonal identity block so bases agree.
                    idsl = (ident[DH:128, DH:128] if vi == 1
                            else ident[0:DH, 0:DH])
                    for yt in range(4):
                        tp = psacc.tile([128, W], F32, tag="acc",
                                        name=f"tp{vi}_{yt}")
                        nc.tensor.transpose(tp[:, 0:DH],
                                            vsrc[:, 128 * yt:128 * (yt + 1)],
                                            idsl)
                        nc.vector.tensor_copy(vbig[:, 4 * vi + yt, 0:DH],
                                              tp[:, 0:DH])

            # ---------- attention: software-pipelined exp-bound loop -------
            ctxu_sb = []  # [128, 512] bf16: rows 0:64 head 2t, 64:128 head 2t+1
            for t in range(NPAIR):
                ctxu_sb.append(qpool.tile([128, W], BF16, tag="ctxu",
                                          name=f"ctxu{t}"))
            wc_sb = [None] * DMT

            qk_tiles = {}
            pr_tiles = {}
            ctxA = [None] * NPAIR
            ctxB = [None] * NPAIR
            norm_state = {}

            def emit_qk(s):
                t, j = divmod(s, NJ)
                qk = psmm.tile([128, 1024], F32, tag="mm", name=f"qk{t}_{j}")
                qpt = qpt_sb[t // 2]
                csl = slice(512 * (t % 2), 512 * (t % 2) + W)
                nc.tensor.matmul(qk[:, 0:W],
                                 kbig[0:DH, 128 * j:128 * (j + 1)],
                                 qpt[0:DH, csl], start=True, stop=True)
                nc.tensor.matmul(qk[:, W:2 * W],
                                 kbig[DH:128, 128 * j:128 * (j + 1)],
                                 qpt[DH:128, csl], start=True, stop=True)
                qk_tiles[s] = qk

            def emit_pv(sv):
                tv, jv = divmod(sv, NJ)
                if jv == 0:
                    ctxA[tv] = psacc.tile([128, W], F32, tag="acc",
                                          name=f"ctxA{tv}")
                    ctxB[tv] = psacc.tile([128, W], F32, tag="acc",
                                          name=f"ctxB{tv}")
                pr = pr_tiles.pop(sv)
                nc.tensor.matmul(ctxA[tv][0:DH + 1, :], vbig[:, jv, 0:DH + 1],
                                 pr[:, 0:W],
                                 start=(jv == 0), stop=(jv == NJ - 1))
                nc.tensor.matmul(ctxB[tv][0:DH + 1, :], vbig[:, jv, 0:DH + 1],
                                 pr[:, W:2 * W],
                                 start=(jv == 0), stop=(jv == NJ - 1))

            def norm_stage(t, stage):
                """staged normalization of pair t (runs during pair t+1)."""
                ns = norm_state.setdefault(t, {})
                if stage == 1:      # denominator rows -> SBUF -> reciprocals
                    denA = npool.tile([1, W], F32, tag="den", name=f"denA{t}")
                    nc.vector.tensor_copy(denA, ctxA[t][DH:DH + 1, :])
                    denB = npool.tile([1, W], F32, tag="den", name=f"denB{t}")
                    nc.vector.tensor_copy(denB, ctxB[t][DH:DH + 1, :])
                    ns['rcpA'] = npool.tile([1, W], F32, tag="rcp",
                                            name=f"rcpA{t}")
                    nc.vector.reciprocal_approx_fast(out=ns['rcpA'], in_=denA)
                    ns['rcpB'] = npool.tile([1, W], F32, tag="rcp",
                                            name=f"rcpB{t}")
                    nc.vector.reciprocal_approx_fast(out=ns['rcpB'], in_=denB)
                elif stage == 2:    # evacuate ctx psum (releases the slots)
                    nc.vector.tensor_copy(ctxu_sb[t][0:DH, :],
                                          ctxA[t][0:DH, :])
                    nc.vector.tensor_copy(ctxu_sb[t][DH:128, :],
                                          ctxB[t][0:DH, :])
                elif stage == 3:    # reciprocals -> DRAM bounce
                    ns['rsc'] = dpool.tile([2, W], F32, tag="rsc",
                                           name=f"rsc{t}")
                    nc.gpsimd.dma_start(out=ns['rsc'][0:1, :], in_=ns['rcpA'])
                    nc.gpsimd.dma_start(out=ns['rsc'][1:2, :], in_=ns['rcpB'])
                elif stage == 4:    # broadcast-expand back to SBUF
                    ns['bc'] = bcpool.tile([128, W], F32, tag="bc",
                                           name=f"bc{t}")
                    rsc = ns['rsc']
                    src = bass.AP(tensor=rsc.tensor, offset=rsc.offset,
                                  ap=[[W, 2], [0, DH], [1, W]])
                    nc.sync.dma_start(out=ns['bc'], in_=src)
                elif stage == 5:    # in-place normalize, head A (gpsimd)
                    nc.gpsimd.tensor_mul(ctxu_sb[t][0:DH, :],
                                         ctxu_sb[t][0:DH, :],
                                         ns['bc'][0:DH, :])
                elif stage == 6:    # in-place normalize, head B (gpsimd)
                    nc.gpsimd.tensor_mul(ctxu_sb[t][DH:128, :],
                                         ctxu_sb[t][DH:128, :],
                                         ns['bc'][DH:128, :])



            # per-exp-step injected work:
            #  - norm(t-1) staged over j==2..10
            #  - QP blocks 2/3 drip-fed during pairs 0/2 at j==5..11 (the
            #    psacc slots are free there: ctx(t-1) was released at j4 and
            #    the qph halves are copied out before ctx(t+1) allocates)
            #  - Wc prefetch issued from the idle gpsimd queue in pair 0
            def injected(t, j):
                if t == 0 and j == 0:
                    emit_vbig_payload()
                if t > 0:
                    stage = {2: 1, 4: 2, 5: 3, 7: 4, 9: 5, 10: 6}.get(j)
                    if stage is not None:
                        norm_stage(t - 1, stage)
                if t in (0, 2):
                    t4 = 2 + t // 2
                    if j in (5, 6, 7, 8):
                        d0 = 2 * (j - 5)
                        emit_qp23_half_mms(t4, 0, [d0, d0 + 1])
                    elif j == 9:
                        emit_qp23_half_mms(t4, 1, [0, 1, 2])
                    elif j == 10:
                        emit_qp23_half_mms(t4, 1, [3, 4, 5])
                    elif j == 11:
                        emit_qp23_half_mms(t4, 1, [6, 7])
                        emit_qp23_half_copy(t4, 0)
                if t in (1, 3) and j == 0:
                    emit_qp23_half_copy(2 + (t - 1) // 2, 1)
                if t == 0 and j == 2:
                    wc_all = wpool.tile([128, DMT, DM], BF16, tag="wc", bufs=1)
                    nc.gpsimd.dma_start(out=wc_all, in_=Wc[:, :, :])
                    for d in range(DMT):
                        wc_sb[d] = wc_all[:, d, :]

            emit_qk(0)
            for s in range(NSTEP + 2):
                if s < NSTEP:
                    t, j = divmod(s, NJ)
                    qk = qk_tiles.pop(s)
                    # last step of the last pair stays on ScalarE (it is idle
                    # by then; keeps the DVE free for the tail normalization)
                    if j in FAST_J and not (t == NPAIR - 1 and j == NJ - 1):
                        pri = ppool.tile([128, 1024], I16, tag="probs",
                                         name=f"pri{t}_{j}")
                        nc.vector.tensor_scalar(
                            out=pri, in0=qk, scalar1=FAST_A, scalar2=FAST_B,
                            op0=ALU.mult, op1=ALU.add)
                        pr_tiles[s] = pri.bitcast(BF16)
                    else:
                        pr = ppool.tile([128, 1024], BF16, tag="probs",
                                        name=f"pr{t}_{j}")
                        nc.scalar.activation(pr, qk, AF.Exp, scale=0.125)
                        pr_tiles[s] = pr
                    if s + 1 < NSTEP:
                        emit_qk(s + 1)
                if s >= 2:
                    emit_pv(s - 2)
                if s < NSTEP:
                    injected(t, j)

            # ---------- tail: pair-7 norm overlapped with out = ctx @ Wc ---
            # he=0..5 accumulate while pair 7's broadcast DMA is in flight;
            # he=6/7 finish each tile once the last ctxu's are normalized.
            def emit_wc(ps, lt, hes, start, stop):
                for half in range(2):
                    for he in hes:
                        nc.tensor.matmul(
                            ps[:, 512 * half:512 * (half + 1)],
                            ctxu_sb[he][:, 128 * lt:128 * (lt + 1)],
                            wc_sb[he][:, 512 * half:512 * (half + 1)],
                            start=(start and he == hes[0]),
                            stop=(stop and he == hes[-1]))

            def emit_out(ps, lt):
                ob = mpool.tile([128, DM], F32, tag="outsb", name=f"ob{lt}")
                nc.vector.tensor_copy(ob, ps)
                nc.sync.dma_start(out=out[128 * lt:128 * (lt + 1), :], in_=ob)

            t7 = NPAIR - 1
            norm_stage(t7, 1)
            norm_stage(t7, 2)
            norm_stage(t7, 3)
            wo = [psmm.tile([128, 1024], F32, tag="mm", name=f"wo{lt}")
                  for lt in range(2)]
            emit_wc(wo[0], 0, [0, 1, 2, 3, 4, 5], True, False)
            norm_stage(t7, 4)
            emit_wc(wo[1], 1, [0, 1, 2, 3, 4, 5], True, False)
            norm_stage(t7, 5)
            norm_stage(t7, 6)
            emit_wc(wo[0], 0, [6, 7], False, True)
            emit_out(wo[0], 0)
            emit_wc(wo[1], 1, [6, 7], False, True)
            emit_out(wo[1], 1)
            for lt in (2, 3):
                ps = psmm.tile([128, 1024], F32, tag="mm", name=f"wo{lt}")
                emit_wc(ps, lt, list(range(DMT)), True, True)
                emit_out(ps, lt)

    nc.compile()
    return nc


_NC = None


def _get_nc():
    global _NC
    if _NC is None:
        _NC = build_nc()
    return _NC


def _pack_rows(x):
    """[1024, N] -> [128, 8, N]: partition p of slice d = row 128*d + p."""
    n = x.shape[1]
    return np.ascontiguousarray(
        x.reshape(DMT, 128, n).transpose(1, 0, 2))


def prepare_in_maps(q, kv, Wq, Wkv, Wc):
    """Host-side prep: transpose + bf16-cast + row-pack, shard q."""
    bf = ml_dtypes.bfloat16
    qT_full = np.asarray(q, np.float32)[0].T.astype(bf)
    kvT = _pack_rows(np.asarray(kv, np.float32)[0].T.astype(bf))
    Wqp = _pack_rows(np.asarray(Wq, np.float32).astype(bf))
    Wkvp = _pack_rows(np.asarray(Wkv, np.float32).astype(bf))
    Wcp = _pack_rows(np.asarray(Wc, np.float32).astype(bf))
    in_maps = []
    for i in range(N_CORES):
        in_maps.append({
            "qT": _pack_rows(qT_full[:, LS * i:LS * (i + 1)]),
            "kvT": kvT,
            "Wq": Wqp,
            "Wkv": Wkvp,
            "Wc": Wcp,
        })
    return in_maps


def kernel(q, kv, Wq, Wkv, Wc, w):
    assert int(w) == W
    q = np.asarray(q, dtype=np.float32)
    assert q.shape[0] == 1 and q.shape[1] == L and q.shape[2] == DM

    in_maps = prepare_in_maps(q, kv, Wq, Wkv, Wc)
    nc = _get_nc()
    res = run_bass_kernel_spmd(nc, in_maps, list(range(N_CORES)))
    out = np.concatenate([res.results[i]["out"] for i in range(N_CORES)],
                         axis=0)
    return out.reshape(1, L, DM).astype(np.float32)


# revision 38
# speedup vs baseline: 1.0341x; 1.0341x over previous
"""Trainium2 Bass kernel for LocalXLAttention (chunk-summed variant).

Math: the reference einsum sums over the chunk index z, so every query
attends to the same three [w, dh] K/V matrices built from chunk sums:
  K_prev = S_k - k_chunk[C-1], K_cur = S_k, K_next = S_k - k_chunk[0]
(identically for V), where S_k = sum_c k_chunk[c].  Per position l, head h:
  attn[l,h,:]  = qp[l,h,:] @ KbigT          (KbigT: [dh, 3w])
  probs        = softmax(attn, axis=-1)
  ctx[l,h,:]   = probs[l,h,:] @ Vbig        (Vbig:  [3w, dh])
  out          = ctx.reshape(L, dm) @ Wc

Sharding: L=4096 split 512 rows/core across 8 cores (data-parallel over
sequence, no collectives).  All inputs are bf16 (host-cast; halves DMA,
same 1-cycle/row PE rate as fp32r).

Schedule notes (engines execute their queues in order, so emission order
IS the schedule):
 - The Scalar engine's exp stream is the critical resource (~1.2us per
   [128,1024] tile, 96 tiles).  The attention loop is software-pipelined:
   QK of step s+1 is emitted right after exp(s); PV runs TWO steps behind
   exp so ctx-psum slot reuse never blocks the PE queue.
 - Prologue PE work is minimized: the kv chunk-sum runs as an in-place
   bf16 tree on the (otherwise idle) DVE, so the S/c0/c7 projections are
   24 matmuls instead of 144; QP head-blocks 0/1 accumulate in the two
   psmm slots during the kv stream; blocks 2/3 are drip-fed into the
   attention loop's PE slack during pairs 0 and 2.
 - Softmax normalization is deferred (an all-ones Vbig column accumulates
   the denominator); per pair: reciprocal_approx_fast on DVE, a
   DRAM-bounce broadcast DMA, and in-place DVE multiplies -- all staged
   across the next pair's steps, nothing on the PE queue.  The last pair
   normalizes via a tiny fp32 K=1 outer-product matmul instead (PE is
   free then and it avoids the DMA round-trip latency in the tail).
"""

import sys
for _p in ('/opt/pypackages', '/opt/trn_rl_repo'):
    if _p not in sys.path:
        sys.path.insert(0, _p)

import math
import numpy as np
import ml_dtypes

import concourse.bass as bass
import concourse.bacc as bacc
import concourse.tile as tile
from concourse import mybir
from concourse.bass_utils import run_bass_kernel_spmd
from concourse.masks import make_identity

F32 = mybir.dt.float32
BF16 = mybir.dt.bfloat16
I16 = mybir.dt.int16
AF = mybir.ActivationFunctionType
ALU = mybir.AluOpType

# Schraudolph fast-exp: bf16 bits of exp(0.125*x) ~= round(A*x + B).
# One DVE tensor_scalar per tile (vs ~1.25us of ScalarE exp); j-chunks in
# FAST_J use it, spreading softmax work across both engines.  Max rel err
# ~3.3% per element, which washes out to <2e-3 in the output (measured).
FAST_J = (3, 7, 11)
FAST_A = 0.125 * 128 / math.log(2)
FAST_B = 127.0 * 128 - 5.5

N_CORES = 8
L = 4096          # full sequence
LS = L // N_CORES # 512 rows per core
DM = 1024
NH = 16
DH = 64
W = 512           # chunk width
C = L // W        # 8 chunks
J3 = 3 * W        # 1536 softmax width
NJ = J3 // 128    # 12 j-chunks
DMT = DM // 128   # 8 dm-chunks
NPAIR = 8         # head pairs
NSTEP = NPAIR * NJ


def build_nc():
    nc = bacc.Bacc(None, target_bir_lowering=False)

    # all inputs host-packed [1024, N] -> [128, 8, N]: partition p of slice d
    # holds source row 128*d + p, so one big-descriptor DMA loads everything
    qT = nc.dram_tensor("qT", [128, DMT, LS], BF16, kind="ExternalInput")
    kvT = nc.dram_tensor("kvT", [128, DMT, L], BF16, kind="ExternalInput")
    Wq = nc.dram_tensor("Wq", [128, DMT, DM], BF16, kind="ExternalInput")
    Wkv = nc.dram_tensor("Wkv", [128, DMT, 2 * DH], BF16,
                         kind="ExternalInput")
    Wc = nc.dram_tensor("Wc", [128, DMT, DM], BF16, kind="ExternalInput")
    out = nc.dram_tensor("out", [LS, DM], F32, kind="ExternalOutput")

    with tile.TileContext(nc) as tc:
        with tc.tile_pool(name="weights", bufs=8) as wpool, \
             tc.tile_pool(name="small", bufs=1) as spool, \
             tc.tile_pool(name="qp", bufs=8) as qpool, \
             tc.tile_pool(name="qpt", bufs=4) as qptpool, \
             tc.tile_pool(name="stream", bufs=1) as stpool, \
             tc.tile_pool(name="ksum", bufs=2) as kspool, \
             tc.tile_pool(name="probs", bufs=6) as ppool, \
             tc.tile_pool(name="norm", bufs=4) as npool, \
             tc.tile_pool(name="bcast", bufs=2) as bcpool, \
             tc.tile_pool(name="misc", bufs=2) as mpool, \
             tc.tile_pool(name="dram", bufs=2, space="DRAM") as dpool, \
             tc.tile_pool(name="psacc", bufs=4, space="PSUM") as psacc, \
             tc.tile_pool(name="psmm", bufs=2, space="PSUM") as psmm:

            # ---------- warm the exp activation table early ----------------
            dummy = spool.tile([1, 8], F32, tag="dummy")
            nc.vector.memset(dummy, 0.0)
            nc.scalar.activation(dummy, dummy, AF.Exp, scale=0.125)

            # ---------- DMA issues: one big transfer per input -------------
            # (16KB-per-partition descriptors move ~3x the bytes/sec of the
            # per-d 2-4KB ones).  Priority: wkv, wq, qt on gpsimd -- the QP
            # matmuls need them early; kv in 4 transfers on sync+scalar.
            wkv_all = wpool.tile([128, DMT, 2 * DH], BF16, tag="wkv", bufs=1)
            nc.gpsimd.dma_start(out=wkv_all, in_=Wkv[:, :, :])
            wq_all = wpool.tile([128, DMT, DM], BF16, tag="wq", bufs=1)
            nc.gpsimd.dma_start(out=wq_all, in_=Wq[:, :, :])
            qt_all = qpool.tile([128, DMT, LS], BF16, tag="qt", bufs=1)
            nc.gpsimd.dma_start(out=qt_all, in_=qT[:, :, :])
            wkv_sb = [wkv_all[:, d, :] for d in range(DMT)]
            wq_sb = [wq_all[:, d, :] for d in range(DMT)]
            qt_sb = [qt_all[:, d, :] for d in range(DMT)]

            ident = spool.tile([128, 128], F32, tag="ident")
            make_identity(nc, ident)
            ones_sb = spool.tile([1, 128], F32, tag="ones")
            nc.vector.memset(ones_sb, 1.0)

            # Vbig shell + its ones (denominator) column, built while DMAs run
            vbig = spool.tile([128, NJ, 68], BF16, tag="vbig")
            ones_col = spool.tile([128, 1], F32, tag="onescol")
            nc.vector.memset(ones_col, 1.0)
            for j in range(NJ):
                nc.vector.tensor_copy(vbig[:, j, DH:DH + 1], ones_col)

            # ---------- kv stream -> DVE chunk-sum tree -> projections -----
            # ps_S = Wkv.T @ (sum_c kv_chunk_c); ps_0/ps_7 = chunk 0/7 proj.
            # rows 0:64 = K, rows 64:128 = V (full-M packed matmuls).
            # QP head-blocks 0,1 ([128,1024] psum each) ride along per-d in
            # the two psmm slots.
            ps_S = psacc.tile([128, W], F32, tag="acc", name="ps_S")
            ps_0 = psacc.tile([128, W], F32, tag="acc", name="ps_0")
            ps_7 = psacc.tile([128, W], F32, tag="acc", name="ps_7")
            qp01_ps = [psmm.tile([128, 1024], F32, tag="mm", name=f"qp_ps{g}")
                       for g in range(2)]

            qp_half_ps = {}
            qpt_sb = [None] * 4

            def emit_qp23_half_mms(t4, half, ds):
                """attention-injected QP head-block (t4 in {2,3}): one
                [128, LS] psacc half, accumulated over the given d's."""
                key = (t4, half)
                if key not in qp_half_ps:
                    qp_half_ps[key] = psacc.tile(
                        [128, W], F32, tag="acc", name=f"qph{t4}_{half}")
                ps = qp_half_ps[key]
                hd = 2 * t4 + half
                for d in ds:
                    nc.tensor.matmul(ps, wq_sb[d][:, 128 * hd:128 * (hd + 1)],
                                     qt_sb[d], start=(d == 0),
                                     stop=(d == DMT - 1))

            def emit_qp23_half_copy(t4, half):
                if qpt_sb[t4] is None:
                    qpt_sb[t4] = qptpool.tile([128, 1024], BF16, tag="qpt",
                                              name=f"qpt{t4}")
                nc.vector.tensor_copy(
                    qpt_sb[t4][:, 512 * half:512 * (half + 1)],
                    qp_half_ps.pop((t4, half)))

            # kv in 4 transfers of 2 d-slices each (16KB/partition); the last
            # rides the gpsimd queue (free once wkv/wq/qt have issued) so
            # three DMA queues carry the stream
            kv_all = stpool.tile([128, DMT, L], BF16, tag="kvstream")
            for g4, eng in enumerate((nc.sync, nc.scalar, nc.sync,
                                      nc.gpsimd)):
                eng.dma_start(out=kv_all[:, 2 * g4:2 * g4 + 2, :],
                              in_=kvT[:, 2 * g4:2 * g4 + 2, :])
            st_sb = [kv_all[:, d, :] for d in range(DMT)]

            # per-d: QP blocks 0/1 first (gated only on wq/qt), then the
            # kv-gated c0/c7 projections, DVE tree-sum, and S projection
            for d in range(DMT):
                st = st_sb[d]
                for g in range(2):
                    for half in range(2):
                        hd = 2 * g + half
                        nc.tensor.matmul(
                            qp01_ps[g][:, 512 * half:512 * (half + 1)],
                            wq_sb[d][:, 128 * hd:128 * (hd + 1)],
                            qt_sb[d], start=(d == 0), stop=(d == DMT - 1))
                nc.tensor.matmul(ps_0, wkv_sb[d], st[:, 0:W],
                                 start=(d == 0), stop=(d == DMT - 1))
                nc.tensor.matmul(ps_7, wkv_sb[d], st[:, L - W:L],
                                 start=(d == 0), stop=(d == DMT - 1))
                if d == DMT - 1:
                    # last d: copy QP block 0 out first (the first QK needs
                    # it), and accumulate the 8 chunks straight into ps_S (8
                    # PE matmuls) instead of waiting on a serial DVE tree
                    qpt_sb[0] = qptpool.tile([128, 1024], BF16, tag="qpt",
                                             name="qpt0")
                    nc.vector.tensor_copy(qpt_sb[0], qp01_ps[0])
                    for c in range(C):
                        nc.tensor.matmul(ps_S, wkv_sb[d],
                                         st[:, W * c:W * (c + 1)],
                                         start=False, stop=(c == C - 1))
                else:
                    # in-place bf16 tree: chunk sum (c0 slice is read by the
                    # ps_0 matmul first; c7 slice is never written)
                    nc.vector.tensor_add(st[:, 0:2048], st[:, 0:2048],
                                         st[:, 2048:4096])
                    nc.vector.tensor_add(st[:, 0:1024], st[:, 0:1024],
                                         st[:, 1024:2048])
                    ks = kspool.tile([128, W], BF16, tag="ks", name=f"ks{d}")
                    nc.vector.tensor_add(ks, st[:, 0:512], st[:, 512:1024])
                    nc.tensor.matmul(ps_S, wkv_sb[d], ks,
                                     start=(d == 0), stop=False)

            # ---------- Kbig [128, 1536] = [prev | cur | next] (bf16) ------
            # emission order feeds the first QK fastest: S copy, prev block,
            # its row-dup (QK j=0..3 reads only columns 0:512), then the rest
            s_sb = spool.tile([128, W], F32, tag="ssb")
            nc.vector.tensor_copy(s_sb, ps_S)
            kbig = spool.tile([128, J3], BF16, tag="kbig")
            nc.vector.tensor_sub(kbig[0:DH, 0:W], s_sb[0:DH, :], ps_7[0:DH, :])
            nc.vector.tensor_copy(kbig[DH:128, 0:W], kbig[0:DH, 0:W])
            nc.vector.tensor_copy(kbig[0:DH, W:2 * W], s_sb[0:DH, :])
            nc.vector.tensor_copy(kbig[DH:128, W:2 * W], kbig[0:DH, W:2 * W])
            nc.vector.tensor_sub(kbig[0:DH, 2 * W:3 * W], s_sb[0:DH, :],
                                 ps_0[0:DH, :])
            nc.vector.tensor_copy(kbig[DH:128, 2 * W:3 * W],
                                  kbig[0:DH, 2 * W:3 * W])
            qpt_sb[1] = qptpool.tile([128, 1024], BF16, tag="qpt",
                                     name="qpt1")
            nc.vector.tensor_copy(qpt_sb[1], qp01_ps[1])

            # V variants in [dh, l] layout (f32, for PE transpose)
            vprev = spool.tile([DH, W], F32, tag="vprev")
            nc.vector.tensor_sub(vprev, s_sb[DH:128, :], ps_7[DH:128, :])
            vnext = spool.tile([DH, W], F32, tag="vnext")
            nc.vector.tensor_sub(vnext, s_sb[DH:128, :], ps_0[DH:128, :])
            vcur = s_sb[DH:128, :]

            # ---------- Vbig payload: 12 PE transposes -> bf16 copies ------
            # (emitted inside the attention loop's first step so the first
            # QK/exp isn't queued behind them; PV first needs vbig at s=2)
            def emit_vbig_payload():
                for vi, vsrc in enumerate((vprev, vcur, vnext)):
                    # vcur is a slice of s_sb at partition base 64; use the
                    # matching diagonal identity block so bases agree.
                    idsl = (ident[DH:128, DH:128] if vi == 1
                            else ident[0:DH, 0:DH])
                    for yt in range(4):
                        tp = psacc.tile([128, W], F32, tag="acc",
                                        name=f"tp{vi}_{yt}")
                        nc.tensor.transpose(tp[:, 0:DH],
                                            vsrc[:, 128 * yt:128 * (yt + 1)],
                                            idsl)
                        nc.vector.tensor_copy(vbig[:, 4 * vi + yt, 0:DH],
                                              tp[:, 0:DH])

            # ---------- attention: software-pipelined exp-bound loop -------
            ctxu_sb = []  # [128, 512] bf16: rows 0:64 head 2t, 64:128 head 2t+1
            for t in range(NPAIR):
                ctxu_sb.append(qpool.tile([128, W], BF16, tag="ctxu",
                                          name=f"ctxu{t}"))
            wc_sb = [None] * DMT

            qk_tiles = {}
            pr_tiles = {}
            ctxA = [None] * NPAIR
            ctxB = [None] * NPAIR
            norm_state = {}

            def emit_qk(s):
                t, j = divmod(s, NJ)
                qk = psmm.tile([128, 1024], F32, tag="mm", name=f"qk{t}_{j}")
                qpt = qpt_sb[t // 2]
                csl = slice(512 * (t % 2), 512 * (t % 2) + W)
                nc.tensor.matmul(qk[:, 0:W],
                                 kbig[0:DH, 128 * j:128 * (j + 1)],
                                 qpt[0:DH, csl], start=True, stop=True)
                nc.tensor.matmul(qk[:, W:2 * W],
                                 kbig[DH:128, 128 * j:128 * (j + 1)],
                                 qpt[DH:128, csl], start=True, stop=True)
                qk_tiles[s] = qk

            def emit_pv(sv):
                tv, jv = divmod(sv, NJ)
                if jv == 0:
                    ctxA[tv] = psacc.tile([128, W], F32, tag="acc",
                                          name=f"ctxA{tv}")
                    ctxB[tv] = psacc.tile([128, W], F32, tag="acc",
                                          name=f"ctxB{tv}")
                pr = pr_tiles.pop(sv)
                nc.tensor.matmul(ctxA[tv][0:DH + 1, :], vbig[:, jv, 0:DH + 1],
                                 pr[:, 0:W],
                                 start=(jv == 0), stop=(jv == NJ - 1))
                nc.tensor.matmul(ctxB[tv][0:DH + 1, :], vbig[:, jv, 0:DH + 1],
                                 pr[:, W:2 * W],
                                 start=(jv == 0), stop=(jv == NJ - 1))

            def norm_stage(t, stage):
                """staged normalization of pair t (runs during pair t+1)."""
                ns = norm_state.setdefault(t, {})
                if stage == 1:      # denominator rows -> SBUF -> reciprocals
                    denA = npool.tile([1, W], F32, tag="den", name=f"denA{t}")
                    nc.vector.tensor_copy(denA, ctxA[t][DH:DH + 1, :])
                    denB = npool.tile([1, W], F32, tag="den", name=f"denB{t}")
                    nc.vector.tensor_copy(denB, ctxB[t][DH:DH + 1, :])
                    ns['rcpA'] = npool.tile([1, W], F32, tag="rcp",
                                            name=f"rcpA{t}")
                    nc.vector.reciprocal_approx_fast(out=ns['rcpA'], in_=denA)
                    ns['rcpB'] = npool.tile([1, W], F32, tag="rcp",
                                            name=f"rcpB{t}")
                    nc.vector.reciprocal_approx_fast(out=ns['rcpB'], in_=denB)
                elif stage == 2:    # evacuate ctx psum (releases the slots)
                    nc.vector.tensor_copy(ctxu_sb[t][0:DH, :],
                                          ctxA[t][0:DH, :])
                    nc.vector.tensor_copy(ctxu_sb[t][DH:128, :],
                                          ctxB[t][0:DH, :])
                elif stage == 3:    # reciprocals -> DRAM bounce
                    ns['rsc'] = dpool.tile([2, W], F32, tag="rsc",
                                           name=f"rsc{t}")
                    nc.gpsimd.dma_start(out=ns['rsc'][0:1, :], in_=ns['rcpA'])
                    nc.gpsimd.dma_start(out=ns['rsc'][1:2, :], in_=ns['rcpB'])
                elif stage == 4:    # broadcast-expand back to SBUF
                    ns['bc'] = bcpool.tile([128, W], F32, tag="bc",
                                           name=f"bc{t}")
                    rsc = ns['rsc']
                    src = bass.AP(tensor=rsc.tensor, offset=rsc.offset,
                                  ap=[[W, 2], [0, DH], [1, W]])
                    nc.sync.dma_start(out=ns['bc'], in_=src)
                elif stage == 5:    # in-place normalize, head A (gpsimd)
                    nc.gpsimd.tensor_mul(ctxu_sb[t][0:DH, :],
                                         ctxu_sb[t][0:DH, :],
                                         ns['bc'][0:DH, :])
                elif stage == 6:    # in-place normalize, head B (gpsimd)
                    nc.gpsimd.tensor_mul(ctxu_sb[t][DH:128, :],
                                         ctxu_sb[t][DH:128, :],
                                         ns['bc'][DH:128, :])



            # per-exp-step injected work:
            #  - norm(t-1) staged over j==2..10
            #  - QP blocks 2/3 drip-fed during pairs 0/2 at j==5..11 (the
            #    psacc slots are free there: ctx(t-1) was released at j4 and
            #    the qph halves are copied out before ctx(t+1) allocates)
            #  - Wc prefetch issued from the idle gpsimd queue in pair 0
            def injected(t, j):
                if t == 0 and j == 0:
                    emit_vbig_payload()
                if t > 0:
                    stage = {2: 1, 4: 2, 5: 3, 7: 4, 9: 5, 10: 6}.get(j)
                    if stage is not None:
                        norm_stage(t - 1, stage)
                if t in (0, 2):
                    t4 = 2 + t // 2
                    if j in (5, 6, 7, 8):
                        d0 = 2 * (j - 5)
                        emit_qp23_half_mms(t4, 0, [d0, d0 + 1])
                    elif j == 9:
                        emit_qp23_half_mms(t4, 1, [0, 1, 2])
                    elif j == 10:
                        emit_qp23_half_mms(t4, 1, [3, 4, 5])
                    elif j == 11:
                        emit_qp23_half_mms(t4, 1, [6, 7])
                        emit_qp23_half_copy(t4, 0)
                if t in (1, 3) and j == 0:
                    emit_qp23_half_copy(2 + (t - 1) // 2, 1)
                if t == 0 and j == 2:
                    wc_all = wpool.tile([128, DMT, DM], BF16, tag="wc", bufs=1)
                    nc.gpsimd.dma_start(out=wc_all, in_=Wc[:, :, :])
                    for d in range(DMT):
                        wc_sb[d] = wc_all[:, d, :]

            emit_qk(0)
            for s in range(NSTEP + 2):
                if s < NSTEP:
                    t, j = divmod(s, NJ)
                    qk = qk_tiles.pop(s)
                    # last step of the last pair stays on ScalarE (it is idle
                    # by then; keeps the DVE free for the tail normalization)
                    if j in FAST_J and not (t == NPAIR - 1 and j == NJ - 1):
                        pri = ppool.tile([128, 1024], I16, tag="probs",
                                         name=f"pri{t}_{j}")
                        nc.vector.tensor_scalar(
                            out=pri, in0=qk, scalar1=FAST_A, scalar2=FAST_B,
                            op0=ALU.mult, op1=ALU.add)
                        pr_tiles[s] = pri.bitcast(BF16)
                    else:
                        pr = ppool.tile([128, 1024], BF16, tag="probs",
                                        name=f"pr{t}_{j}")
                        nc.scalar.activation(pr, qk, AF.Exp, scale=0.125)
                        pr_tiles[s] = pr
                    if s + 1 < NSTEP:
                        emit_qk(s + 1)
                if s >= 2:
                    emit_pv(s - 2)
                if s < NSTEP:
                    injected(t, j)

            # ---------- tail: pair-7 norm overlapped with out = ctx @ Wc ---
            # he=0..5 accumulate while pair 7's broadcast DMA is in flight;
            # he=6/7 finish each tile once the last ctxu's are normalized.
            def emit_wc(ps, lt, hes, start, stop):
                for half in range(2):
                    for he in hes:
                        nc.tensor.matmul(
                            ps[:, 512 * half:512 * (half + 1)],
                            ctxu_sb[he][:, 128 * lt:128 * (lt + 1)],
                            wc_sb[he][:, 512 * half:512 * (half + 1)],
                            start=(start and he == hes[0]),
                            stop=(stop and he == hes[-1]))

            def emit_out(ps, lt):
                ob = mpool.tile([128, DM], F32, tag="outsb", name=f"ob{lt}")
                nc.vector.tensor_copy(ob, ps)
                nc.sync.dma_start(out=out[128 * lt:128 * (lt + 1), :], in_=ob)

            # pair-7 norm: recips+copies on DVE, then K=1 fp32 outer-product
            # matmuls broadcast the reciprocals (no DRAM round-trip latency);
            # he=0..5 of the first two Wc tiles overlap the DVE chain.
            t7 = NPAIR - 1
            norm_stage(t7, 1)
            norm_stage(t7, 2)
            wo = [psmm.tile([128, 1024], F32, tag="mm", name=f"wo{lt}")
                  for lt in range(2)]
            emit_wc(wo[0], 0, [0, 1, 2, 3, 4, 5], True, False)
            ns7 = norm_state[t7]
            bcA7 = psacc.tile([128, W], F32, tag="acc", name="bcA7")
            nc.tensor.matmul(bcA7, ones_sb, ns7['rcpA'], start=True, stop=True)
            bcB7 = psacc.tile([128, W], F32, tag="acc", name="bcB7")
            nc.tensor.matmul(bcB7, ones_sb, ns7['rcpB'], start=True, stop=True)
            emit_wc(wo[1], 1, [0, 1, 2, 3, 4, 5], True, False)
            nc.vector.tensor_mul(ctxu_sb[t7][0:DH, :],
                                 ctxu_sb[t7][0:DH, :], bcA7[0:DH, :])
            nc.vector.tensor_mul(ctxu_sb[t7][DH:128, :],
                                 ctxu_sb[t7][DH:128, :], bcB7[0:DH, :])
            emit_wc(wo[0], 0, [6, 7], False, True)
            emit_out(wo[0], 0)
            emit_wc(wo[1], 1, [6, 7], False, True)
            emit_out(wo[1], 1)
            for lt in (2, 3):
                ps = psmm.tile([128, 1024], F32, tag="mm", name=f"wo{lt}")
                emit_wc(ps, lt, list(range(DMT)), True, True)
                emit_out(ps, lt)

    nc.compile()
    return nc


_NC = None


def _get_nc():
    global _NC
    if _NC is None:
        _NC = build_nc()
    return _NC


def _pack_rows(x):
    """[1024, N] -> [128, 8, N]: partition p of slice d = row 128*d + p."""
    n = x.shape[1]
    return np.ascontiguousarray(
        x.reshape(DMT, 128, n).transpose(1, 0, 2))


def prepare_in_maps(q, kv, Wq, Wkv, Wc):
    """Host-side prep: transpose + bf16-cast + row-pack, shard q."""
    bf = ml_dtypes.bfloat16
    qT_full = np.asarray(q, np.float32)[0].T.astype(bf)
    kvT = _pack_rows(np.asarray(kv, np.float32)[0].T.astype(bf))
    Wqp = _pack_rows(np.asarray(Wq, np.float32).astype(bf))
    Wkvp = _pack_rows(np.asarray(Wkv, np.float32).astype(bf))
    Wcp = _pack_rows(np.asarray(Wc, np.float32).astype(bf))
    in_maps = []
    for i in range(N_CORES):
        in_maps.append({
            "qT": _pack_rows(qT_full[:, LS * i:LS * (i + 1)]),
            "kvT": kvT,
            "Wq": Wqp,
            "Wkv": Wkvp,
            "Wc": Wcp,
        })
    return in_maps


def kernel(q, kv, Wq, Wkv, Wc, w):
    assert int(w) == W
    q = np.asarray(q, dtype=np.float32)
    assert q.shape[0] == 1 and q.shape[1] == L and q.shape[2] == DM

    in_maps = prepare_in_maps(q, kv, Wq, Wkv, Wc)
    nc = _get_nc()
    res = run_bass_kernel_spmd(nc, in_maps, list(range(N_CORES)))
    out = np.concatenate([res.results[i]["out"] for i in range(N_CORES)],
                         axis=0)
    return out.reshape(1, L, DM).astype(np.float32)


# revision 39
# speedup vs baseline: 1.0374x; 1.0033x over previous
"""Trainium2 Bass kernel for LocalXLAttention (chunk-summed variant).

Math: the reference einsum sums over the chunk index z, so every query
attends to the same three [w, dh] K/V matrices built from chunk sums:
  K_prev = S_k - k_chunk[C-1], K_cur = S_k, K_next = S_k - k_chunk[0]
(identically for V), where S_k = sum_c k_chunk[c].  Per position l, head h:
  attn[l,h,:]  = qp[l,h,:] @ KbigT          (KbigT: [dh, 3w])
  probs        = softmax(attn, axis=-1)
  ctx[l,h,:]   = probs[l,h,:] @ Vbig        (Vbig:  [3w, dh])
  out          = ctx.reshape(L, dm) @ Wc

Sharding: L=4096 split 512 rows/core across 8 cores (data-parallel over
sequence, no collectives).  All inputs are bf16 (host-cast; halves DMA,
same 1-cycle/row PE rate as fp32r).

Schedule notes (engines execute their queues in order, so emission order
IS the schedule):
 - The Scalar engine's exp stream is the critical resource (~1.2us per
   [128,1024] tile, 96 tiles).  The attention loop is software-pipelined:
   QK of step s+1 is emitted right after exp(s); PV runs TWO steps behind
   exp so ctx-psum slot reuse never blocks the PE queue.
 - Prologue PE work is minimized: the kv chunk-sum runs as an in-place
   bf16 tree on the (otherwise idle) DVE, so the S/c0/c7 projections are
   24 matmuls instead of 144; QP head-blocks 0/1 accumulate in the two
   psmm slots during the kv stream; blocks 2/3 are drip-fed into the
   attention loop's PE slack during pairs 0 and 2.
 - Softmax normalization is deferred (an all-ones Vbig column accumulates
   the denominator); per pair: reciprocal_approx_fast on DVE, a
   DRAM-bounce broadcast DMA, and in-place DVE multiplies -- all staged
   across the next pair's steps, nothing on the PE queue.  The last pair
   normalizes via a tiny fp32 K=1 outer-product matmul instead (PE is
   free then and it avoids the DMA round-trip latency in the tail).
"""

import sys
for _p in ('/opt/pypackages', '/opt/trn_rl_repo'):
    if _p not in sys.path:
        sys.path.insert(0, _p)

import math
import numpy as np
import ml_dtypes

import concourse.bass as bass
import concourse.bacc as bacc
import concourse.tile as tile
from concourse import mybir
from concourse.bass_utils import run_bass_kernel_spmd
from concourse.masks import make_identity

F32 = mybir.dt.float32
BF16 = mybir.dt.bfloat16
I16 = mybir.dt.int16
AF = mybir.ActivationFunctionType
ALU = mybir.AluOpType

# Schraudolph fast-exp: bf16 bits of exp(0.125*x) ~= round(A*x + B).
# One DVE tensor_scalar per tile (vs ~1.25us of ScalarE exp); j-chunks in
# FAST_J use it, spreading softmax work across both engines.  Max rel err
# ~3.3% per element, which washes out to <2e-3 in the output (measured).
FAST_J = (3, 7, 11)
FAST_A = 0.125 * 128 / math.log(2)
FAST_B = 127.0 * 128 - 5.5

N_CORES = 8
L = 4096          # full sequence
LS = L // N_CORES # 512 rows per core
DM = 1024
NH = 16
DH = 64
W = 512           # chunk width
C = L // W        # 8 chunks
J3 = 3 * W        # 1536 softmax width
NJ = J3 // 128    # 12 j-chunks
DMT = DM // 128   # 8 dm-chunks
NPAIR = 8         # head pairs
NSTEP = NPAIR * NJ


def build_nc():
    nc = bacc.Bacc(None, target_bir_lowering=False)

    # all inputs host-packed [1024, N] -> [128, 8, N]: partition p of slice d
    # holds source row 128*d + p, so one big-descriptor DMA loads everything
    qT = nc.dram_tensor("qT", [128, DMT, LS], BF16, kind="ExternalInput")
    kvT = nc.dram_tensor("kvT", [128, DMT, L], BF16, kind="ExternalInput")
    Wq = nc.dram_tensor("Wq", [128, DMT, DM], BF16, kind="ExternalInput")
    Wkv = nc.dram_tensor("Wkv", [128, DMT, 2 * DH], BF16,
                         kind="ExternalInput")
    Wc = nc.dram_tensor("Wc", [128, DMT, DM], BF16, kind="ExternalInput")
    out = nc.dram_tensor("out", [LS, DM], F32, kind="ExternalOutput")

    with tile.TileContext(nc) as tc:
        with tc.tile_pool(name="weights", bufs=8) as wpool, \
             tc.tile_pool(name="small", bufs=1) as spool, \
             tc.tile_pool(name="qp", bufs=8) as qpool, \
             tc.tile_pool(name="qpt", bufs=4) as qptpool, \
             tc.tile_pool(name="stream", bufs=1) as stpool, \
             tc.tile_pool(name="ksum", bufs=2) as kspool, \
             tc.tile_pool(name="probs", bufs=6) as ppool, \
             tc.tile_pool(name="norm", bufs=4) as npool, \
             tc.tile_pool(name="bcast", bufs=2) as bcpool, \
             tc.tile_pool(name="misc", bufs=2) as mpool, \
             tc.tile_pool(name="dram", bufs=2, space="DRAM") as dpool, \
             tc.tile_pool(name="psacc", bufs=4, space="PSUM") as psacc, \
             tc.tile_pool(name="psmm", bufs=2, space="PSUM") as psmm:

            # ---------- warm the exp activation table early ----------------
            dummy = spool.tile([1, 8], F32, tag="dummy")
            nc.vector.memset(dummy, 0.0)
            nc.scalar.activation(dummy, dummy, AF.Exp, scale=0.125)

            # ---------- DMA issues: one big transfer per input -------------
            # (16KB-per-partition descriptors move ~3x the bytes/sec of the
            # per-d 2-4KB ones).  Priority: wkv, wq, qt on gpsimd -- the QP
            # matmuls need them early; kv in 4 transfers on sync+scalar.
            wkv_all = wpool.tile([128, DMT, 2 * DH], BF16, tag="wkv", bufs=1)
            nc.gpsimd.dma_start(out=wkv_all, in_=Wkv[:, :, :])
            wq_all = wpool.tile([128, DMT, DM], BF16, tag="wq", bufs=1)
            nc.gpsimd.dma_start(out=wq_all, in_=Wq[:, :, :])
            qt_all = qpool.tile([128, DMT, LS], BF16, tag="qt", bufs=1)
            nc.gpsimd.dma_start(out=qt_all, in_=qT[:, :, :])
            wkv_sb = [wkv_all[:, d, :] for d in range(DMT)]
            wq_sb = [wq_all[:, d, :] for d in range(DMT)]
            qt_sb = [qt_all[:, d, :] for d in range(DMT)]

            ident = spool.tile([128, 128], F32, tag="ident")
            make_identity(nc, ident)
            ones_sb = spool.tile([1, 128], F32, tag="ones")
            nc.vector.memset(ones_sb, 1.0)

            # Vbig shell + its ones (denominator) column, built while DMAs run
            vbig = spool.tile([128, NJ, 68], BF16, tag="vbig")
            ones_col = spool.tile([128, 1], F32, tag="onescol")
            nc.vector.memset(ones_col, 1.0)
            for j in range(NJ):
                nc.vector.tensor_copy(vbig[:, j, DH:DH + 1], ones_col)

            # ---------- kv stream -> DVE chunk-sum tree -> projections -----
            # ps_S = Wkv.T @ (sum_c kv_chunk_c); ps_0/ps_7 = chunk 0/7 proj.
            # rows 0:64 = K, rows 64:128 = V (full-M packed matmuls).
            # QP head-blocks 0,1 ([128,1024] psum each) ride along per-d in
            # the two psmm slots.
            ps_S = psacc.tile([128, W], F32, tag="acc", name="ps_S")
            ps_0 = psacc.tile([128, W], F32, tag="acc", name="ps_0")
            ps_7 = psacc.tile([128, W], F32, tag="acc", name="ps_7")
            qp01_ps = [psmm.tile([128, 1024], F32, tag="mm", name=f"qp_ps{g}")
                       for g in range(2)]

            qp_half_ps = {}
            qpt_sb = [None] * 4

            def emit_qp23_half_mms(t4, half, ds):
                """attention-injected QP head-block (t4 in {2,3}): one
                [128, LS] psacc half, accumulated over the given d's."""
                key = (t4, half)
                if key not in qp_half_ps:
                    qp_half_ps[key] = psacc.tile(
                        [128, W], F32, tag="acc", name=f"qph{t4}_{half}")
                ps = qp_half_ps[key]
                hd = 2 * t4 + half
                for d in ds:
                    nc.tensor.matmul(ps, wq_sb[d][:, 128 * hd:128 * (hd + 1)],
                                     qt_sb[d], start=(d == 0),
                                     stop=(d == DMT - 1))

            def emit_qp23_half_copy(t4, half):
                if qpt_sb[t4] is None:
                    qpt_sb[t4] = qptpool.tile([128, 1024], BF16, tag="qpt",
                                              name=f"qpt{t4}")
                nc.vector.tensor_copy(
                    qpt_sb[t4][:, 512 * half:512 * (half + 1)],
                    qp_half_ps.pop((t4, half)))

            # kv as 8 per-d transfers (8KB/partition descriptors, many in
            # flight -- the combination that measures fastest), spread over
            # sync/scalar; the last two ride gpsimd, which frees up right
            # around when they are needed
            kv_all = stpool.tile([128, DMT, L], BF16, tag="kvstream")
            kv_eng = (nc.sync, nc.scalar, nc.sync, nc.scalar,
                      nc.sync, nc.scalar, nc.gpsimd, nc.gpsimd)
            for d in range(DMT):
                kv_eng[d].dma_start(out=kv_all[:, d, :], in_=kvT[:, d, :])
            st_sb = [kv_all[:, d, :] for d in range(DMT)]

            # per-d: QP blocks 0/1 first (gated only on wq/qt), then the
            # kv-gated c0/c7 projections, DVE tree-sum, and S projection
            for d in range(DMT):
                st = st_sb[d]
                for g in range(2):
                    for half in range(2):
                        hd = 2 * g + half
                        nc.tensor.matmul(
                            qp01_ps[g][:, 512 * half:512 * (half + 1)],
                            wq_sb[d][:, 128 * hd:128 * (hd + 1)],
                            qt_sb[d], start=(d == 0), stop=(d == DMT - 1))
                nc.tensor.matmul(ps_0, wkv_sb[d], st[:, 0:W],
                                 start=(d == 0), stop=(d == DMT - 1))
                nc.tensor.matmul(ps_7, wkv_sb[d], st[:, L - W:L],
                                 start=(d == 0), stop=(d == DMT - 1))
                if d == DMT - 1:
                    # last d: copy QP block 0 out first (the first QK needs
                    # it), and accumulate the 8 chunks straight into ps_S (8
                    # PE matmuls) instead of waiting on a serial DVE tree
                    qpt_sb[0] = qptpool.tile([128, 1024], BF16, tag="qpt",
                                             name="qpt0")
                    nc.vector.tensor_copy(qpt_sb[0], qp01_ps[0])
                    for c in range(C):
                        nc.tensor.matmul(ps_S, wkv_sb[d],
                                         st[:, W * c:W * (c + 1)],
                                         start=False, stop=(c == C - 1))
                else:
                    # in-place bf16 tree: chunk sum (c0 slice is read by the
                    # ps_0 matmul first; c7 slice is never written)
                    nc.vector.tensor_add(st[:, 0:2048], st[:, 0:2048],
                                         st[:, 2048:4096])
                    nc.vector.tensor_add(st[:, 0:1024], st[:, 0:1024],
                                         st[:, 1024:2048])
                    ks = kspool.tile([128, W], BF16, tag="ks", name=f"ks{d}")
                    nc.vector.tensor_add(ks, st[:, 0:512], st[:, 512:1024])
                    nc.tensor.matmul(ps_S, wkv_sb[d], ks,
                                     start=(d == 0), stop=False)

            # ---------- Kbig [128, 1536] = [prev | cur | next] (bf16) ------
            # emission order feeds the first QK fastest: S copy, prev block,
            # its row-dup (QK j=0..3 reads only columns 0:512), then the rest
            s_sb = spool.tile([128, W], F32, tag="ssb")
            nc.vector.tensor_copy(s_sb, ps_S)
            kbig = spool.tile([128, J3], BF16, tag="kbig")
            nc.vector.tensor_sub(kbig[0:DH, 0:W], s_sb[0:DH, :], ps_7[0:DH, :])
            nc.vector.tensor_copy(kbig[DH:128, 0:W], kbig[0:DH, 0:W])
            nc.vector.tensor_copy(kbig[0:DH, W:2 * W], s_sb[0:DH, :])
            nc.vector.tensor_copy(kbig[DH:128, W:2 * W], kbig[0:DH, W:2 * W])
            nc.vector.tensor_sub(kbig[0:DH, 2 * W:3 * W], s_sb[0:DH, :],
                                 ps_0[0:DH, :])
            nc.vector.tensor_copy(kbig[DH:128, 2 * W:3 * W],
                                  kbig[0:DH, 2 * W:3 * W])
            qpt_sb[1] = qptpool.tile([128, 1024], BF16, tag="qpt",
                                     name="qpt1")
            nc.vector.tensor_copy(qpt_sb[1], qp01_ps[1])

            # V variants in [dh, l] layout (f32, for PE transpose)
            vprev = spool.tile([DH, W], F32, tag="vprev")
            nc.vector.tensor_sub(vprev, s_sb[DH:128, :], ps_7[DH:128, :])
            vnext = spool.tile([DH, W], F32, tag="vnext")
            nc.vector.tensor_sub(vnext, s_sb[DH:128, :], ps_0[DH:128, :])
            vcur = s_sb[DH:128, :]

            # ---------- Vbig payload: 12 PE transposes -> bf16 copies ------
            # (emitted inside the attention loop's first step so the first
            # QK/exp isn't queued behind them; PV first needs vbig at s=2)
            def emit_vbig_payload():
                for vi, vsrc in enumerate((vprev, vcur, vnext)):
                    # vcur is a slice of s_sb at partition base 64; use the
                    # matching diagonal identity block so bases agree.
                    idsl = (ident[DH:128, DH:128] if vi == 1
                            else ident[0:DH, 0:DH])
                    for yt in range(4):
                        tp = psacc.tile([128, W], F32, tag="acc",
                                        name=f"tp{vi}_{yt}")
                        nc.tensor.transpose(tp[:, 0:DH],
                                            vsrc[:, 128 * yt:128 * (yt + 1)],
                                            idsl)
                        nc.vector.tensor_copy(vbig[:, 4 * vi + yt, 0:DH],
                                              tp[:, 0:DH])

            # ---------- attention: software-pipelined exp-bound loop -------
            ctxu_sb = []  # [128, 512] bf16: rows 0:64 head 2t, 64:128 head 2t+1
            for t in range(NPAIR):
                ctxu_sb.append(qpool.tile([128, W], BF16, tag="ctxu",
                                          name=f"ctxu{t}"))
            wc_sb = [None] * DMT

            qk_tiles = {}
            pr_tiles = {}
            ctxA = [None] * NPAIR
            ctxB = [None] * NPAIR
            norm_state = {}

            def emit_qk(s):
                t, j = divmod(s, NJ)
                qk = psmm.tile([128, 1024], F32, tag="mm", name=f"qk{t}_{j}")
                qpt = qpt_sb[t // 2]
                csl = slice(512 * (t % 2), 512 * (t % 2) + W)
                nc.tensor.matmul(qk[:, 0:W],
                                 kbig[0:DH, 128 * j:128 * (j + 1)],
                                 qpt[0:DH, csl], start=True, stop=True)
                nc.tensor.matmul(qk[:, W:2 * W],
                                 kbig[DH:128, 128 * j:128 * (j + 1)],
                                 qpt[DH:128, csl], start=True, stop=True)
                qk_tiles[s] = qk

            def emit_pv(sv):
                tv, jv = divmod(sv, NJ)
                if jv == 0:
                    ctxA[tv] = psacc.tile([128, W], F32, tag="acc",
                                          name=f"ctxA{tv}")
                    ctxB[tv] = psacc.tile([128, W], F32, tag="acc",
                                          name=f"ctxB{tv}")
                pr = pr_tiles.pop(sv)
                nc.tensor.matmul(ctxA[tv][0:DH + 1, :], vbig[:, jv, 0:DH + 1],
                                 pr[:, 0:W],
                                 start=(jv == 0), stop=(jv == NJ - 1))
                nc.tensor.matmul(ctxB[tv][0:DH + 1, :], vbig[:, jv, 0:DH + 1],
                                 pr[:, W:2 * W],
                                 start=(jv == 0), stop=(jv == NJ - 1))

            def norm_stage(t, stage):
                """staged normalization of pair t (runs during pair t+1)."""
                ns = norm_state.setdefault(t, {})
                if stage == 1:      # denominator rows -> SBUF -> reciprocals
                    denA = npool.tile([1, W], F32, tag="den", name=f"denA{t}")
                    nc.vector.tensor_copy(denA, ctxA[t][DH:DH + 1, :])
                    denB = npool.tile([1, W], F32, tag="den", name=f"denB{t}")
                    nc.vector.tensor_copy(denB, ctxB[t][DH:DH + 1, :])
                    ns['rcpA'] = npool.tile([1, W], F32, tag="rcp",
                                            name=f"rcpA{t}")
                    nc.vector.reciprocal_approx_fast(out=ns['rcpA'], in_=denA)
                    ns['rcpB'] = npool.tile([1, W], F32, tag="rcp",
                                            name=f"rcpB{t}")
                    nc.vector.reciprocal_approx_fast(out=ns['rcpB'], in_=denB)
                elif stage == 2:    # evacuate ctx psum (releases the slots)
                    nc.vector.tensor_copy(ctxu_sb[t][0:DH, :],
                                          ctxA[t][0:DH, :])
                    nc.vector.tensor_copy(ctxu_sb[t][DH:128, :],
                                          ctxB[t][0:DH, :])
                elif stage == 3:    # reciprocals -> DRAM bounce
                    ns['rsc'] = dpool.tile([2, W], F32, tag="rsc",
                                           name=f"rsc{t}")
                    nc.gpsimd.dma_start(out=ns['rsc'][0:1, :], in_=ns['rcpA'])
                    nc.gpsimd.dma_start(out=ns['rsc'][1:2, :], in_=ns['rcpB'])
                elif stage == 4:    # broadcast-expand back to SBUF
                    ns['bc'] = bcpool.tile([128, W], F32, tag="bc",
                                           name=f"bc{t}")
                    rsc = ns['rsc']
                    src = bass.AP(tensor=rsc.tensor, offset=rsc.offset,
                                  ap=[[W, 2], [0, DH], [1, W]])
                    nc.sync.dma_start(out=ns['bc'], in_=src)
                elif stage == 5:    # in-place normalize, head A (gpsimd)
                    nc.gpsimd.tensor_mul(ctxu_sb[t][0:DH, :],
                                         ctxu_sb[t][0:DH, :],
                                         ns['bc'][0:DH, :])
                elif stage == 6:    # in-place normalize, head B (gpsimd)
                    nc.gpsimd.tensor_mul(ctxu_sb[t][DH:128, :],
                                         ctxu_sb[t][DH:128, :],
                                         ns['bc'][DH:128, :])



            # per-exp-step injected work:
            #  - norm(t-1) staged over j==2..10
            #  - QP blocks 2/3 drip-fed during pairs 0/2 at j==5..11 (the
            #    psacc slots are free there: ctx(t-1) was released at j4 and
            #    the qph halves are copied out before ctx(t+1) allocates)
            #  - Wc prefetch issued from the idle gpsimd queue in pair 0
            def injected(t, j):
                if t == 0 and j == 0:
                    emit_vbig_payload()
                if t > 0:
                    stage = {2: 1, 4: 2, 5: 3, 7: 4, 9: 5, 10: 6}.get(j)
                    if stage is not None:
                        norm_stage(t - 1, stage)
                if t in (0, 2):
                    t4 = 2 + t // 2
                    if j in (5, 6, 7, 8):
                        d0 = 2 * (j - 5)
                        emit_qp23_half_mms(t4, 0, [d0, d0 + 1])
                    elif j == 9:
                        emit_qp23_half_mms(t4, 1, [0, 1, 2])
                    elif j == 10:
                        emit_qp23_half_mms(t4, 1, [3, 4, 5])
                    elif j == 11:
                        emit_qp23_half_mms(t4, 1, [6, 7])
                        emit_qp23_half_copy(t4, 0)
                if t in (1, 3) and j == 0:
                    emit_qp23_half_copy(2 + (t - 1) // 2, 1)
                if t == 0 and j == 2:
                    wc_all = wpool.tile([128, DMT, DM], BF16, tag="wc", bufs=1)
                    nc.gpsimd.dma_start(out=wc_all, in_=Wc[:, :, :])
                    for d in range(DMT):
                        wc_sb[d] = wc_all[:, d, :]

            emit_qk(0)
            for s in range(NSTEP + 2):
                if s < NSTEP:
                    t, j = divmod(s, NJ)
                    qk = qk_tiles.pop(s)
                    # last step of the last pair stays on ScalarE (it is idle
                    # by then; keeps the DVE free for the tail normalization)
                    if j in FAST_J and not (t == NPAIR - 1 and j == NJ - 1):
                        pri = ppool.tile([128, 1024], I16, tag="probs",
                                         name=f"pri{t}_{j}")
                        nc.vector.tensor_scalar(
                            out=pri, in0=qk, scalar1=FAST_A, scalar2=FAST_B,
                            op0=ALU.mult, op1=ALU.add)
                        pr_tiles[s] = pri.bitcast(BF16)
                    else:
                        pr = ppool.tile([128, 1024], BF16, tag="probs",
                                        name=f"pr{t}_{j}")
                        nc.scalar.activation(pr, qk, AF.Exp, scale=0.125)
                        pr_tiles[s] = pr
                    if s + 1 < NSTEP:
                        emit_qk(s + 1)
                if s >= 2:
                    emit_pv(s - 2)
                if s < NSTEP:
                    injected(t, j)

            # ---------- tail: pair-7 norm overlapped with out = ctx @ Wc ---
            # he=0..5 accumulate while pair 7's broadcast DMA is in flight;
            # he=6/7 finish each tile once the last ctxu's are normalized.
            def emit_wc(ps, lt, hes, start, stop):
                for half in range(2):
                    for he in hes:
                        nc.tensor.matmul(
                            ps[:, 512 * half:512 * (half + 1)],
                            ctxu_sb[he][:, 128 * lt:128 * (lt + 1)],
                            wc_sb[he][:, 512 * half:512 * (half + 1)],
                            start=(start and he == hes[0]),
                            stop=(stop and he == hes[-1]))

            def emit_out(ps, lt):
                ob = mpool.tile([128, DM], F32, tag="outsb", name=f"ob{lt}")
                nc.vector.tensor_copy(ob, ps)
                nc.sync.dma_start(out=out[128 * lt:128 * (lt + 1), :], in_=ob)

            # pair-7 norm: recips+copies on DVE, then K=1 fp32 outer-product
            # matmuls broadcast the reciprocals (no DRAM round-trip latency);
            # he=0..5 of the first two Wc tiles overlap the DVE chain.
            t7 = NPAIR - 1
            norm_stage(t7, 1)
            norm_stage(t7, 2)
            wo = [psmm.tile([128, 1024], F32, tag="mm", name=f"wo{lt}")
                  for lt in range(2)]
            emit_wc(wo[0], 0, [0, 1, 2, 3, 4, 5], True, False)
            ns7 = norm_state[t7]
            bcA7 = psacc.tile([128, W], F32, tag="acc", name="bcA7")
            nc.tensor.matmul(bcA7, ones_sb, ns7['rcpA'], start=True, stop=True)
            bcB7 = psacc.tile([128, W], F32, tag="acc", name="bcB7")
            nc.tensor.matmul(bcB7, ones_sb, ns7['rcpB'], start=True, stop=True)
            emit_wc(wo[1], 1, [0, 1, 2, 3, 4, 5], True, False)
            nc.vector.tensor_mul(ctxu_sb[t7][0:DH, :],
                                 ctxu_sb[t7][0:DH, :], bcA7[0:DH, :])
            nc.vector.tensor_mul(ctxu_sb[t7][DH:128, :],
                                 ctxu_sb[t7][DH:128, :], bcB7[0:DH, :])
            emit_wc(wo[0], 0, [6, 7], False, True)
            emit_out(wo[0], 0)
            emit_wc(wo[1], 1, [6, 7], False, True)
            emit_out(wo[1], 1)
            for lt in (2, 3):
                ps = psmm.tile([128, 1024], F32, tag="mm", name=f"wo{lt}")
                emit_wc(ps, lt, list(range(DMT)), True, True)
                emit_out(ps, lt)

    nc.compile()
    return nc


_NC = None


def _get_nc():
    global _NC
    if _NC is None:
        _NC = build_nc()
    return _NC


def _pack_rows(x):
    """[1024, N] -> [128, 8, N]: partition p of slice d = row 128*d + p."""
    n = x.shape[1]
    return np.ascontiguousarray(
        x.reshape(DMT, 128, n).transpose(1, 0, 2))


def prepare_in_maps(q, kv, Wq, Wkv, Wc):
    """Host-side prep: transpose + bf16-cast + row-pack, shard q."""
    bf = ml_dtypes.bfloat16
    qT_full = np.asarray(q, np.float32)[0].T.astype(bf)
    kvT = _pack_rows(np.asarray(kv, np.float32)[0].T.astype(bf))
    Wqp = _pack_rows(np.asarray(Wq, np.float32).astype(bf))
    Wkvp = _pack_rows(np.asarray(Wkv, np.float32).astype(bf))
    Wcp = _pack_rows(np.asarray(Wc, np.float32).astype(bf))
    in_maps = []
    for i in range(N_CORES):
        in_maps.append({
            "qT": _pack_rows(qT_full[:, LS * i:LS * (i + 1)]),
            "kvT": kvT,
            "Wq": Wqp,
            "Wkv": Wkvp,
            "Wc": Wcp,
        })
    return in_maps


def kernel(q, kv, Wq, Wkv, Wc, w):
    assert int(w) == W
    q = np.asarray(q, dtype=np.float32)
    assert q.shape[0] == 1 and q.shape[1] == L and q.shape[2] == DM

    in_maps = prepare_in_maps(q, kv, Wq, Wkv, Wc)
    nc = _get_nc()
    res = run_bass_kernel_spmd(nc, in_maps, list(range(N_CORES)))
    out = np.concatenate([res.results[i]["out"] for i in range(N_CORES)],
                         axis=0)
    return out.reshape(1, L, DM).astype(np.float32)
